# revision 15
# baseline (speedup 1.0000x reference)
"""Trainium2 Bass kernel v8 for IntervalClusterTriplet (hard-mining triplet loss).

Math: loss = mean_i relu(sqrt(max_{j in cluster(i)} d2_ij)
                       - sqrt(min_{j not in cluster(i)} d2_ij) + 1)
with d2_ij = n_i + n_j - 2 e_i.e_j. Only max/min VALUES are needed; n_i is
added per-partition after the reduce.

v8 design (vs v7: 90us):
 - ONE fp8 DoubleRow matmul pass produces partial_ij = -2e_i.e_j + n_j
   directly in PSUM: k-tile0 carries fp8(-2e_i) x fp8(e_j), k-tile1 carries
   ones x (n_j split into 3 fp8 rows hi/lo/lo2, residual < 0.01). This kills
   v7's second norm-accumulation matmul pass (PE 54us -> 14us) and the
   norm-broadcast adds on DVE.
 - Per 128-row chunk the 8192 PSUM cols are consumed by two engines in
   parallel (PSUM is readable only by ACT and DVE; at most one PSUM operand
   per instruction):
     * dve: custom ADD_MAX/ADD_MIN vs. the constant cluster mask on the
       128-wide diag block; custom MIN_MIN pairs (one PSUM group + one
       fp16-evacuated SBUF group per instruction) for groups 1..7.
     * act: evacuates groups 1,3,5 and half of 7 to fp16 (pair fodder), and
       retires group 0's non-diag columns via a fused exp-accumulate
       softmin: S = sum exp((SHIFT - d2)/tau), min ~= SHIFT - tau ln S.
       tau=3.0 keeps the softmin bias ~1e-5 of the loss while the exponent
       window stays in fp32 range (bias = (SHIFT - n_i)/tau per partition).
 - Epilogue (once, [128,8]-wide): combine exact mins + softmin, add n_i,
   sqrt via exp(0.5 ln x) so the WHOLE program uses one ACT table set
   (natural_log_exp_and_others: Exp, Ln, Copy) -> single table load.
Numpy simulation of this exact pipeline: rel err ~1e-5 (tolerance 2e-2).

Sharding: rows of the distance matrix across 8 cores (1024 rows each); each
core gets E^T rolled so its own 1024 columns come first (one SPMD program).
Per-core output is the partial loss sum; host adds and divides by N.
"""

import numpy as np
import ml_dtypes

import concourse.bacc as bacc
import concourse.mybir as mybir
import concourse.tile as tile
from concourse.bass_utils import run_bass_kernel_spmd

from concourse.dve_spec import Spec, Src0, Src1, C0, minn, maxx, lower
from concourse.dve_uop import DveOpSpec
import concourse.dve_ops as dops

C, S, D = 1024, 8, 128
N = C * S              # 8192 embeddings
CORES = 8
M = N // CORES         # 1024 rows per core
P = 128                # partitions (rows per chunk)
CH = M // P            # 8 chunks per core
GW = 1024              # group width (2 PSUM banks)
TN = 512               # matmul moving width (1 PSUM bank)
TAU = 3.0
SHIFT = 64.0
BIG = 1.0e30
F32 = mybir.dt.float32
F32R = mybir.dt.float32r
F16 = mybir.dt.float16
BF16 = mybir.dt.bfloat16
F8 = mybir.dt.float8e4
ALU = mybir.AluOpType
AX = mybir.AxisListType
ACT = mybir.ActivationFunctionType
PM = mybir.MatmulPerfMode

_CACHE: dict = {}


def _ref_red(body_fn, red_fn):
    def _r(in0, in1, c0, c1, c2):
        b = body_fn(np.asarray(in0, np.float32),
                    np.asarray(in1, np.float32)).astype(np.float32)
        acc = red_fn(c0, b.reshape(b.shape[0], -1), red_fn)
        return b, acc
    return _r


def _red_min(c0, b, _):
    return np.minimum(c0, b.min(axis=-1, keepdims=True))


def _red_max(c0, b, _):
    return np.maximum(c0, b.max(axis=-1, keepdims=True))


def _register_op(name, body, accum, body_fn, red_fn):
    """Register a custom DVE table op (idempotent across re-imports)."""
    for o in dops.OPS:
        if o.name == name:
            return o
    spec = Spec(body=body, accum=accum, accum_init=C0,
                reference=_ref_red(body_fn, red_fn))
    op = dops.DveOp(name, spec, subdim=False, uops_sha={})
    dops.OPS.append(op)
    dops.CUSTOM_DVE_SPECS[name] = spec
    dops._SUB_OPCODE_FOR_NAME[name] = dops._CUSTOM_DVE_ROW_BASE + len(dops.OPS) - 1
    for ver in ("v3", "v4"):
        s = DveOpSpec(name=name, opcode=dops.get_dve_sub_opcode(name),
                      uops=lower(spec, ver=ver), rd1_en=True)
        op.uops_sha[ver] = s.sha(ver)
    return op


ADD_MIN = _register_op("ANT_ADD_MIN_RED", Src0 + Src1, minn,
                       lambda a, b: a + b, _red_min)
ADD_MAX = _register_op("ANT_ADD_MAX_RED", Src0 + Src1, maxx,
                       lambda a, b: a + b, _red_max)
MIN_MIN = _register_op("ANT_MIN_MIN_RED", minn(Src0, Src1), minn,
                       lambda a, b: np.minimum(a, b), _red_min)


def _unify_act_tables():
    """Keep Exp/Ln/Copy resolvable ONLY via natural_log_exp_and_others so
    every ACT instruction uses one table set and a single InstLoadActFuncSet
    is hoisted to program start. Set positions are preserved; only
    membership shrinks."""
    if getattr(bacc, "_ant_act_tables_unified_v8", False):
        return
    orig = bacc.get_activation_tables
    A = mybir.ActivationFunctionType

    def patched(arch):
        tables = orig(arch)
        for name, funcs in tables.items():
            if name != "natural_log_exp_and_others" and isinstance(funcs, set):
                funcs.discard(A.Copy)
                funcs.discard(A.Exp)
                funcs.discard(A.Ln)
        return tables

    bacc.get_activation_tables = patched
    bacc._ant_act_tables_unified_v8 = True


_unify_act_tables()


def build_program(reps: int = 1):
    nc = bacc.Bacc("TRN2", target_bir_lowering=False, debug=False)
    et_d = nc.dram_tensor("et8", [P, 2 * N], F8, kind="ExternalInput").ap()
    em_d = nc.dram_tensor("em8", [P, 2 * M], F8, kind="ExternalInput").ap()
    m8max_d = nc.dram_tensor("m8max", [P, P], F32, kind="ExternalInput").ap()
    m8min_d = nc.dram_tensor("m8min", [P, P], F32, kind="ExternalInput").ap()
    nbias_d = nc.dram_tensor("nbias", [P, CH], F32, kind="ExternalInput").ap()
    nmy_d = nc.dram_tensor("nmy", [P, CH], F32, kind="ExternalInput").ap()
    onesc_d = nc.dram_tensor("onesc", [P, 2], F32R, kind="ExternalInput").ap()
    out_d = nc.dram_tensor("out", [1, 1], F32, kind="ExternalOutput").ap()

    def body(tc, cin, work, stg, sth, pg):
        # ---- input DMAs. SP pays ~0.5us of descriptor time per dma_start,
        # serially, so: few large DMAs, small/early-needed tensors first,
        # and the big et chunks issued from the otherwise-idle Pool engine
        # so SP and Pool generate descriptors in parallel.
        em = cin.tile([P, 2, M], F8, tag="em")
        nc.sync.dma_start(em[:, 0, :], em_d[:, 0:M])
        nc.sync.dma_start(em[:, 1, :], em_d[:, M:2 * M])
        et = cin.tile([P, 2, N], F8, tag="et")
        for c in range(4):
            nc.gpsimd.dma_start(et[:, 0, c * 2048:(c + 1) * 2048],
                                et_d[:, c * 2048:(c + 1) * 2048])
        nc.gpsimd.dma_start(et[:, 1, :], et_d[:, N:2 * N])
        m8max = cin.tile([P, P], F32, tag="m8max")
        nc.sync.dma_start(m8max, m8max_d)
        m8min = cin.tile([P, P], F32, tag="m8min")
        nc.sync.dma_start(m8min, m8min_d)
        nbias = cin.tile([P, CH], F32, tag="nbias")
        nc.sync.dma_start(nbias, nbias_d)
        # needed only by the epilogue: issue last on SP
        nmy = cin.tile([P, CH], F32, tag="nmy")
        nc.sync.dma_start(nmy, nmy_d)
        ones_c = cin.tile([P, 2], F32R, tag="ones_c")
        nc.sync.dma_start(ones_c, onesc_d)

        # ---- accumulator tiles (ACT/custom accum_out overwrites: no init)
        sacc = work.tile([P, CH], F32, tag="sacc")
        # exact mins, slot-major layout: slot k for chunk m at [:, k*CH+m]
        mc = work.tile([P, 5 * CH], F32, tag="mc")
        apm = work.tile([P, CH], F32, tag="apm")
        dummy = work.tile([P, 1], BF16, tag="dummy")

        def mm(pt, lo, hi, lhs, gbase):
            for h in range((hi - lo) // TN):
                nc.tensor.matmul(pt[:, lo + h * TN:lo + (h + 1) * TN],
                                 lhsT=lhs,
                                 rhs=et[:, :, gbase + lo + h * TN:
                                        gbase + lo + (h + 1) * TN],
                                 start=True, stop=True,
                                 perf_mode=PM.DoubleRow)

        # ---- main loop over 8 row chunks.
        # The tile scheduler is a per-engine ready-heap keyed by program
        # issue order, so ACT emission order here IS its execution order
        # among ready instructions: evacs first (they feed DVE pairs), the
        # slow exp softmin LAST (its p0 tile sits in a dedicated PSUM slot
        # so holding it never blocks the pair pipeline). The g1 evac is
        # software-pipelined one chunk ahead so pair1 fires at chunk start.
        ND = GW - P  # 896

        def lhs_of(m):
            return em[:, :, m * P:(m + 1) * P]

        # prologue: chunk 0's g1 evac (pair fodder ready at chunk start)
        pa = pg.tile([P, GW], F32, tag="pg")
        mm(pa, 0, GW, lhs_of(0), 1 * GW)
        s1 = stg.tile([P, GW], F16, tag="sg")
        nc.scalar.copy(s1, pa)

        for m in range(CH):
            lhs = lhs_of(m)
            dcol = m * P

            # groups 2..6: pair psum group with the previous fp16 evac
            sa = s1
            for q in range(3):
                gb = 2 + 2 * q
                pb = pg.tile([P, GW], F32, tag="pg")
                mm(pb, 0, GW, lhs, gb * GW)
                nc.vector._custom_dve(
                    MIN_MIN, out=dummy.broadcast_to(pb.shape),
                    in0=pb, in1=sa, s0=3.0e38,
                    accum_out=mc[:, (1 + q) * CH + m:(1 + q) * CH + m + 1])
                if q < 2:
                    pa = pg.tile([P, GW], F32, tag="pg")
                    mm(pa, 0, GW, lhs, (3 + 2 * q) * GW)
                    sa = stg.tile([P, GW], F16, tag="sg")
                    nc.scalar.copy(sa, pa)

            # group 7: evac first half to fp16, pair with second half
            p7 = pg.tile([P, GW], F32, tag="pg")
            mm(p7, 0, GW, lhs, 7 * GW)
            sh = sth.tile([P, TN], F16, tag="sh")
            nc.scalar.copy(sh, p7[:, 0:TN])
            nc.vector._custom_dve(
                MIN_MIN, out=dummy.broadcast_to(p7[:, TN:GW].shape),
                in0=p7[:, TN:GW], in1=sh, s0=3.0e38,
                accum_out=mc[:, 4 * CH + m:4 * CH + m + 1])

            # g0 LAST: own columns. The matmuls PACK the non-diag cols to
            # p0[:, 0:896] and park the diag block at p0[:, 896:1024], so
            # ACT retires the non-diag part in ONE exp-accum softmin pass
            # emitted after every evac (ACT ready-heap = emission order).
            # Matmul splits respect PSUM bank edges (out stays in one bank).
            p0 = pg.tile([P, GW], F32, tag="pg")
            pieces = []
            bounds = sorted({0, TN, ND, dcol} - {GW})
            for lo, hi in zip(bounds, bounds[1:] + [ND]):
                if hi > lo:
                    pieces.append((lo, hi, lo if lo < dcol else lo + P))
            pieces.append((ND, GW, dcol))
            # start=True zeroes the whole 2KB PSUM bank: only the first
            # piece per bank starts; later pieces accumulate onto the zeros.
            by_bank = {}
            for lo, hi, src in pieces:
                by_bank.setdefault(lo // TN, []).append((lo, hi, src))
            for bank, plist in by_bank.items():
                for i, (lo, hi, src) in enumerate(plist):
                    nc.tensor.matmul(p0[:, lo:hi], lhsT=lhs,
                                     rhs=et[:, :, src:src + (hi - lo)],
                                     start=(i == 0), stop=(i == len(plist) - 1),
                                     perf_mode=PM.DoubleRow,
                                     skip_group_check=True)
            nc.vector._custom_dve(
                ADD_MAX, out=dummy.broadcast_to(p0[:, ND:GW].shape),
                in0=p0[:, ND:GW], in1=m8max, s0=-3.0e38,
                accum_out=apm[:, m:m + 1])
            nc.vector._custom_dve(
                ADD_MIN, out=dummy.broadcast_to(p0[:, ND:GW].shape),
                in0=p0[:, ND:GW], in1=m8min, s0=3.0e38,
                accum_out=mc[:, m:m + 1])
            nc.scalar.activation(
                dummy.broadcast_to(p0[:, 0:ND].shape), p0[:, 0:ND],
                ACT.Exp, bias=nbias[:, m:m + 1], scale=-1.0 / TAU,
                accum_out=sacc[:, m:m + 1])

            # software pipeline: next chunk's g1 evac (last in ACT heap)
            if m + 1 < CH:
                pa = pg.tile([P, GW], F32, tag="pg")
                mm(pa, 0, GW, lhs_of(m + 1), 1 * GW)
                s1 = stg.tile([P, GW], F16, tag="sg")
                nc.scalar.copy(s1, pa)



        # ---- epilogue, [128, CH]-wide
        lnS = work.tile([P, CH], F32, tag="lnS")
        nc.scalar.activation(lnS, sacc, ACT.Ln)
        soft = work.tile([P, CH], F32, tag="soft")
        nc.vector.tensor_scalar(soft, lnS, -TAU, SHIFT,
                                op0=ALU.mult, op1=ALU.add)

        mp = work.tile([P, CH], F32, tag="mp")
        nc.vector.tensor_tensor(mp, mc[:, 0:CH], mc[:, CH:2 * CH], op=ALU.min)
        for k in (2, 3, 4):
            nc.vector.tensor_tensor(mp, mp, mc[:, k * CH:(k + 1) * CH],
                                    op=ALU.min)
        exact = work.tile([P, CH], F32, tag="exact")
        nc.vector.tensor_add(exact, mp, nmy)

        sq = work.tile([P, 2 * CH], F32, tag="sq")
        nc.vector.tensor_add(sq[:, 0:CH], apm, nmy)
        nc.vector.tensor_tensor(sq[:, CH:2 * CH], soft, exact, op=ALU.min)

        # sqrt(x) = exp(0.5 ln x): stays on the single ACT table set
        lsq = work.tile([P, 2 * CH], F32, tag="lsq")
        nc.scalar.activation(lsq, sq, ACT.Ln)
        rt = work.tile([P, 2 * CH], F32, tag="rt")
        nc.scalar.activation(rt, lsq, ACT.Exp, scale=0.5)

        diff = work.tile([P, CH], F32, tag="diff")
        nc.vector.tensor_sub(diff, rt[:, 0:CH], rt[:, CH:2 * CH])
        lterm = work.tile([P, CH], F32, tag="lterm")
        nc.vector.tensor_scalar(lterm, diff, 1.0, 0.0,
                                op0=ALU.add, op1=ALU.max)

        lsum = work.tile([P, 1], F32R, tag="lsum")
        with nc.allow_low_precision(reason="f32r rounding of per-row loss ok"):
            nc.vector.tensor_reduce(lsum, lterm, axis=AX.X, op=ALU.add)
        pf = pg.tile([P, GW], F32, tag="pg")
        nc.tensor.matmul(pf[0:1, 0:2], lhsT=lsum, rhs=ones_c, start=True,
                         stop=True)
        outsb = work.tile([1, 1], F32, tag="outsb")
        nc.scalar.copy(outsb, pf[0:1, 0:1])
        nc.sync.dma_start(out_d, outsb)

    with tile.TileContext(nc) as tc:
        with (
            tc.tile_pool(name="cin", bufs=2) as cin,
            tc.tile_pool(name="work", bufs=2) as work,
            tc.tile_pool(name="stg", bufs=6) as stg,
            tc.tile_pool(name="sth", bufs=2) as sth,
            tc.tile_pool(name="pg", bufs=4, space="PSUM") as pg,
        ):
            args = (tc, cin, work, stg, sth, pg)
            if reps == 1:
                body(*args)
            else:
                # For_i puts an all-engine barrier at each iteration, which
                # re-serializes the ~8us DMA lead-in + drain every rep.
                # Unroll the body so the barrier amortizes across UNROLL
                # reps; pools rotate tiles across body instances, so
                # consecutive instances overlap barrier-free.
                unroll = 4 if reps % 4 == 0 else (2 if reps % 2 == 0 else 1)
                with tc.For_i(0, reps // unroll, 1):
                    for _ in range(unroll):
                        body(*args)

    nc.compile()
    return nc


def _q8(x):
    return np.asarray(np.asarray(x, np.float32), ml_dtypes.float8_e4m3)


def make_in_maps(batch: np.ndarray):
    E = np.ascontiguousarray(batch.reshape(N, D).astype(np.float32, copy=False))
    n = (E.astype(np.float64) * E).sum(1).astype(np.float32)
    hi = _q8(n).astype(np.float32)
    lo = _q8(n - hi).astype(np.float32)
    lo2 = _q8(n - hi - lo)
    k0_full = _q8(E.T)            # [D, N]
    em_full = _q8(-2.0 * E.T)     # [D, N]

    idx = np.arange(P)
    same = (idx[:, None] // S) == (idx[None, :] // S)
    m8min = np.where(same, BIG, 0.0).astype(np.float32)   # exclude cluster
    m8max = np.where(same, 0.0, -BIG).astype(np.float32)  # keep cluster

    in_maps = []
    for r in range(CORES):
        order = (np.arange(N) + r * M) % N
        et8 = np.zeros((P, 2 * N), dtype=ml_dtypes.float8_e4m3)
        et8[:, 0:N] = k0_full[:, order]
        et8[0, N:2 * N] = _q8(hi[order])
        et8[1, N:2 * N] = _q8(lo[order])
        et8[2, N:2 * N] = lo2[order]
        em8 = np.zeros((P, 2 * M), dtype=ml_dtypes.float8_e4m3)
        em8[:, 0:M] = em_full[:, r * M:(r + 1) * M]
        em8[0:3, M:2 * M] = np.asarray(1.0, ml_dtypes.float8_e4m3)
        nmy = n[r * M:(r + 1) * M].reshape(CH, P).T.copy()   # [P, CH]
        nbias = ((SHIFT - nmy) / TAU).astype(np.float32)
        in_maps.append({"et8": et8, "em8": em8, "m8max": m8max,
                        "m8min": m8min, "nbias": nbias, "nmy": nmy,
                        "onesc": np.ones((P, 2), np.float32)})
    return in_maps


def kernel(batch: np.ndarray) -> np.ndarray:
    if "nc" not in _CACHE:
        _CACHE["nc"] = build_program(reps=1)
    nc = _CACHE["nc"]
    in_maps = make_in_maps(np.asarray(batch))
    res = run_bass_kernel_spmd(nc, in_maps, core_ids=list(range(CORES)))
    total = sum(float(res.results[r]["out"][0, 0]) for r in range(CORES))
    return np.float32(total / N)


# revision 16
# speedup vs baseline: 1.0543x; 1.0543x over previous
"""Trainium2 Bass kernel v8 for IntervalClusterTriplet (hard-mining triplet loss).

Math: loss = mean_i relu(sqrt(max_{j in cluster(i)} d2_ij)
                       - sqrt(min_{j not in cluster(i)} d2_ij) + 1)
with d2_ij = n_i + n_j - 2 e_i.e_j. Only max/min VALUES are needed; n_i is
added per-partition after the reduce.

v8 design (vs v7: 90us):
 - ONE fp8 DoubleRow matmul pass produces partial_ij = -2e_i.e_j + n_j
   directly in PSUM: k-tile0 carries fp8(-2e_i) x fp8(e_j), k-tile1 carries
   ones x (n_j split into 3 fp8 rows hi/lo/lo2, residual < 0.01). This kills
   v7's second norm-accumulation matmul pass (PE 54us -> 14us) and the
   norm-broadcast adds on DVE.
 - Per 128-row chunk the 8192 PSUM cols are consumed by two engines in
   parallel (PSUM is readable only by ACT and DVE; at most one PSUM operand
   per instruction):
     * dve: custom ADD_MAX/ADD_MIN vs. the constant cluster mask on the
       128-wide diag block; custom MIN_MIN pairs (one PSUM group + one
       fp16-evacuated SBUF group per instruction) for groups 1..7.
     * act: evacuates groups 1,3,5 and half of 7 to fp16 (pair fodder), and
       retires group 0's non-diag columns via a fused exp-accumulate
       softmin: S = sum exp((SHIFT - d2)/tau), min ~= SHIFT - tau ln S.
       tau=3.0 keeps the softmin bias ~1e-5 of the loss while the exponent
       window stays in fp32 range (bias = (SHIFT - n_i)/tau per partition).
 - Epilogue (once, [128,8]-wide): combine exact mins + softmin, add n_i,
   sqrt via exp(0.5 ln x) so the WHOLE program uses one ACT table set
   (natural_log_exp_and_others: Exp, Ln, Copy) -> single table load.
Numpy simulation of this exact pipeline: rel err ~1e-5 (tolerance 2e-2).

Sharding: rows of the distance matrix across 8 cores (1024 rows each); each
core gets E^T rolled so its own 1024 columns come first (one SPMD program).
Per-core output is the partial loss sum; host adds and divides by N.
"""

import numpy as np
import ml_dtypes

import concourse.bacc as bacc
import concourse.mybir as mybir
import concourse.tile as tile
from concourse.bass_utils import run_bass_kernel_spmd

from concourse.dve_spec import Spec, Src0, Src1, C0, minn, maxx, lower
from concourse.dve_uop import DveOpSpec
import concourse.dve_ops as dops

C, S, D = 1024, 8, 128
N = C * S              # 8192 embeddings
CORES = 8
M = N // CORES         # 1024 rows per core
P = 128                # partitions (rows per chunk)
CH = M // P            # 8 chunks per core
GW = 1024              # group width (2 PSUM banks)
TN = 512               # matmul moving width (1 PSUM bank)
TAU = 3.0
SHIFT = 64.0
BIG = 1.0e30
F32 = mybir.dt.float32
F32R = mybir.dt.float32r
F16 = mybir.dt.float16
BF16 = mybir.dt.bfloat16
F8 = mybir.dt.float8e4
ALU = mybir.AluOpType
AX = mybir.AxisListType
ACT = mybir.ActivationFunctionType
PM = mybir.MatmulPerfMode

_CACHE: dict = {}


def _ref_red(body_fn, red_fn):
    def _r(in0, in1, c0, c1, c2):
        b = body_fn(np.asarray(in0, np.float32),
                    np.asarray(in1, np.float32)).astype(np.float32)
        acc = red_fn(c0, b.reshape(b.shape[0], -1), red_fn)
        return b, acc
    return _r


def _red_min(c0, b, _):
    return np.minimum(c0, b.min(axis=-1, keepdims=True))


def _red_max(c0, b, _):
    return np.maximum(c0, b.max(axis=-1, keepdims=True))


def _register_op(name, body, accum, body_fn, red_fn):
    """Register a custom DVE table op (idempotent across re-imports)."""
    for o in dops.OPS:
        if o.name == name:
            return o
    spec = Spec(body=body, accum=accum, accum_init=C0,
                reference=_ref_red(body_fn, red_fn))
    op = dops.DveOp(name, spec, subdim=False, uops_sha={})
    dops.OPS.append(op)
    dops.CUSTOM_DVE_SPECS[name] = spec
    dops._SUB_OPCODE_FOR_NAME[name] = dops._CUSTOM_DVE_ROW_BASE + len(dops.OPS) - 1
    for ver in ("v3", "v4"):
        s = DveOpSpec(name=name, opcode=dops.get_dve_sub_opcode(name),
                      uops=lower(spec, ver=ver), rd1_en=True)
        op.uops_sha[ver] = s.sha(ver)
    return op


ADD_MIN = _register_op("ANT_ADD_MIN_RED", Src0 + Src1, minn,
                       lambda a, b: a + b, _red_min)
ADD_MAX = _register_op("ANT_ADD_MAX_RED", Src0 + Src1, maxx,
                       lambda a, b: a + b, _red_max)
MIN_MIN = _register_op("ANT_MIN_MIN_RED", minn(Src0, Src1), minn,
                       lambda a, b: np.minimum(a, b), _red_min)


def _unify_act_tables():
    """Keep Exp/Ln/Copy resolvable ONLY via natural_log_exp_and_others so
    every ACT instruction uses one table set and a single InstLoadActFuncSet
    is hoisted to program start. Set positions are preserved; only
    membership shrinks."""
    if getattr(bacc, "_ant_act_tables_unified_v8", False):
        return
    orig = bacc.get_activation_tables
    A = mybir.ActivationFunctionType

    def patched(arch):
        tables = orig(arch)
        for name, funcs in tables.items():
            if name != "natural_log_exp_and_others" and isinstance(funcs, set):
                funcs.discard(A.Copy)
                funcs.discard(A.Exp)
                funcs.discard(A.Ln)
        return tables

    bacc.get_activation_tables = patched
    bacc._ant_act_tables_unified_v8 = True


_unify_act_tables()


def build_program(reps: int = 1):
    nc = bacc.Bacc("TRN2", target_bir_lowering=False, debug=False)
    et_d = nc.dram_tensor("et8", [P, 2 * N], F8, kind="ExternalInput").ap()
    em_d = nc.dram_tensor("em8", [P, 2 * M], F8, kind="ExternalInput").ap()
    m8max_d = nc.dram_tensor("m8max", [P, P], F32, kind="ExternalInput").ap()
    m8min_d = nc.dram_tensor("m8min", [P, P], F32, kind="ExternalInput").ap()
    nbias_d = nc.dram_tensor("nbias", [P, CH], F32, kind="ExternalInput").ap()
    nmy_d = nc.dram_tensor("nmy", [P, CH], F32, kind="ExternalInput").ap()
    onesc_d = nc.dram_tensor("onesc", [P, 2], F32R, kind="ExternalInput").ap()
    out_d = nc.dram_tensor("out", [1, 1], F32, kind="ExternalOutput").ap()

    def body(tc, cin, work, stg, sth, pg):
        # ---- input DMAs. SP pays ~0.5us of descriptor time per dma_start,
        # serially, so: few large DMAs, small/early-needed tensors first,
        # and the big et chunks issued from the otherwise-idle Pool engine
        # so SP and Pool generate descriptors in parallel.
        em = cin.tile([P, 2, M], F8, tag="em")
        nc.sync.dma_start(em[:, 0, :], em_d[:, 0:M])
        nc.sync.dma_start(em[:, 1, :], em_d[:, M:2 * M])
        et = cin.tile([P, 2, N], F8, tag="et")
        for c in range(4):
            nc.gpsimd.dma_start(et[:, 0, c * 2048:(c + 1) * 2048],
                                et_d[:, c * 2048:(c + 1) * 2048])
        nc.gpsimd.dma_start(et[:, 1, :], et_d[:, N:2 * N])
        m8max = cin.tile([P, P], F32, tag="m8max")
        nc.sync.dma_start(m8max, m8max_d)
        m8min = cin.tile([P, P], F32, tag="m8min")
        nc.sync.dma_start(m8min, m8min_d)
        nbias = cin.tile([P, CH], F32, tag="nbias")
        nc.sync.dma_start(nbias, nbias_d)
        # needed only by the epilogue: issue last on SP
        nmy = cin.tile([P, CH], F32, tag="nmy")
        nc.sync.dma_start(nmy, nmy_d)
        ones_c = cin.tile([P, 2], F32R, tag="ones_c")
        nc.sync.dma_start(ones_c, onesc_d)

        # ---- accumulator tiles (ACT/custom accum_out overwrites: no init)
        sacc = work.tile([P, CH], F32, tag="sacc")
        # exact mins, slot-major layout: slot k for chunk m at [:, k*CH+m]
        mc = work.tile([P, 5 * CH], F32, tag="mc")
        apm = work.tile([P, CH], F32, tag="apm")
        dummy = work.tile([P, 1], BF16, tag="dummy")

        def mm(pt, lo, hi, lhs, gbase):
            for h in range((hi - lo) // TN):
                nc.tensor.matmul(pt[:, lo + h * TN:lo + (h + 1) * TN],
                                 lhsT=lhs,
                                 rhs=et[:, :, gbase + lo + h * TN:
                                        gbase + lo + (h + 1) * TN],
                                 start=True, stop=True,
                                 perf_mode=PM.DoubleRow)

        # ---- main loop over 8 row chunks.
        # The tile scheduler is a per-engine ready-heap keyed by program
        # issue order, so ACT emission order here IS its execution order
        # among ready instructions: evacs first (they feed DVE pairs), the
        # slow exp softmin LAST (its p0 tile sits in a dedicated PSUM slot
        # so holding it never blocks the pair pipeline). The g1 evac is
        # software-pipelined one chunk ahead so pair1 fires at chunk start.
        ND = GW - P  # 896

        def lhs_of(m):
            return em[:, :, m * P:(m + 1) * P]

        for m in range(CH):
            lhs = lhs_of(m)
            dcol = m * P

            # g1 evac: first in the ACT ready-heap for this chunk
            pa = pg.tile([P, GW], F32, tag="pg")
            mm(pa, 0, GW, lhs, 1 * GW)
            sa = stg.tile([P, GW], F16, tag="sg")
            nc.scalar.copy(sa, pa)

            # groups 2..6: pair psum group with the previous fp16 evac
            for q in range(3):
                gb = 2 + 2 * q
                pb = pg.tile([P, GW], F32, tag="pg")
                mm(pb, 0, GW, lhs, gb * GW)
                nc.vector._custom_dve(
                    MIN_MIN, out=dummy.broadcast_to(pb.shape),
                    in0=pb, in1=sa, s0=3.0e38,
                    accum_out=mc[:, (1 + q) * CH + m:(1 + q) * CH + m + 1])
                if q < 2:
                    pa = pg.tile([P, GW], F32, tag="pg")
                    mm(pa, 0, GW, lhs, (3 + 2 * q) * GW)
                    sa = stg.tile([P, GW], F16, tag="sg")
                    nc.scalar.copy(sa, pa)

            # group 7: evac first half to fp16, pair with second half
            p7 = pg.tile([P, GW], F32, tag="pg")
            mm(p7, 0, GW, lhs, 7 * GW)
            sh = sth.tile([P, TN], F16, tag="sh")
            nc.scalar.copy(sh, p7[:, 0:TN])
            nc.vector._custom_dve(
                MIN_MIN, out=dummy.broadcast_to(p7[:, TN:GW].shape),
                in0=p7[:, TN:GW], in1=sh, s0=3.0e38,
                accum_out=mc[:, 4 * CH + m:4 * CH + m + 1])

            # g0 LAST: own columns. The matmuls PACK the non-diag cols to
            # p0[:, 0:896] and park the diag block at p0[:, 896:1024], so
            # ACT retires the non-diag part in ONE exp-accum softmin pass
            # emitted after every evac (ACT ready-heap = emission order).
            # Matmul splits respect PSUM bank edges (out stays in one bank).
            p0 = pg.tile([P, GW], F32, tag="pg")
            pieces = []
            bounds = sorted({0, TN, ND, dcol} - {GW})
            for lo, hi in zip(bounds, bounds[1:] + [ND]):
                if hi > lo:
                    pieces.append((lo, hi, lo if lo < dcol else lo + P))
            pieces.append((ND, GW, dcol))
            # start=True zeroes the whole 2KB PSUM bank: only the first
            # piece per bank starts; later pieces accumulate onto the zeros.
            by_bank = {}
            for lo, hi, src in pieces:
                by_bank.setdefault(lo // TN, []).append((lo, hi, src))
            for bank, plist in by_bank.items():
                for i, (lo, hi, src) in enumerate(plist):
                    nc.tensor.matmul(p0[:, lo:hi], lhsT=lhs,
                                     rhs=et[:, :, src:src + (hi - lo)],
                                     start=(i == 0), stop=(i == len(plist) - 1),
                                     perf_mode=PM.DoubleRow,
                                     skip_group_check=True)
            nc.vector._custom_dve(
                ADD_MAX, out=dummy.broadcast_to(p0[:, ND:GW].shape),
                in0=p0[:, ND:GW], in1=m8max, s0=-3.0e38,
                accum_out=apm[:, m:m + 1])
            nc.vector._custom_dve(
                ADD_MIN, out=dummy.broadcast_to(p0[:, ND:GW].shape),
                in0=p0[:, ND:GW], in1=m8min, s0=3.0e38,
                accum_out=mc[:, m:m + 1])
            nc.scalar.activation(
                dummy.broadcast_to(p0[:, 0:ND].shape), p0[:, 0:ND],
                ACT.Exp, bias=nbias[:, m:m + 1], scale=-1.0 / TAU,
                accum_out=sacc[:, m:m + 1])



        # ---- epilogue, [128, CH]-wide
        lnS = work.tile([P, CH], F32, tag="lnS")
        nc.scalar.activation(lnS, sacc, ACT.Ln)
        soft = work.tile([P, CH], F32, tag="soft")
        nc.vector.tensor_scalar(soft, lnS, -TAU, SHIFT,
                                op0=ALU.mult, op1=ALU.add)

        mp = work.tile([P, CH], F32, tag="mp")
        nc.vector.tensor_tensor(mp, mc[:, 0:CH], mc[:, CH:2 * CH], op=ALU.min)
        for k in (2, 3, 4):
            nc.vector.tensor_tensor(mp, mp, mc[:, k * CH:(k + 1) * CH],
                                    op=ALU.min)
        exact = work.tile([P, CH], F32, tag="exact")
        nc.vector.tensor_add(exact, mp, nmy)

        sq = work.tile([P, 2 * CH], F32, tag="sq")
        nc.vector.tensor_add(sq[:, 0:CH], apm, nmy)
        nc.vector.tensor_tensor(sq[:, CH:2 * CH], soft, exact, op=ALU.min)

        # sqrt(x) = exp(0.5 ln x): stays on the single ACT table set
        lsq = work.tile([P, 2 * CH], F32, tag="lsq")
        nc.scalar.activation(lsq, sq, ACT.Ln)
        rt = work.tile([P, 2 * CH], F32, tag="rt")
        nc.scalar.activation(rt, lsq, ACT.Exp, scale=0.5)

        diff = work.tile([P, CH], F32, tag="diff")
        nc.vector.tensor_sub(diff, rt[:, 0:CH], rt[:, CH:2 * CH])
        lterm = work.tile([P, CH], F32, tag="lterm")
        nc.vector.tensor_scalar(lterm, diff, 1.0, 0.0,
                                op0=ALU.add, op1=ALU.max)

        lsum = work.tile([P, 1], F32R, tag="lsum")
        with nc.allow_low_precision(reason="f32r rounding of per-row loss ok"):
            nc.vector.tensor_reduce(lsum, lterm, axis=AX.X, op=ALU.add)
        pf = pg.tile([P, GW], F32, tag="pg")
        nc.tensor.matmul(pf[0:1, 0:2], lhsT=lsum, rhs=ones_c, start=True,
                         stop=True)
        outsb = work.tile([1, 1], F32, tag="outsb")
        nc.scalar.copy(outsb, pf[0:1, 0:1])
        nc.sync.dma_start(out_d, outsb)

    with tile.TileContext(nc) as tc:
        with (
            tc.tile_pool(name="cin", bufs=2) as cin,
            tc.tile_pool(name="work", bufs=2) as work,
            tc.tile_pool(name="stg", bufs=6) as stg,
            tc.tile_pool(name="sth", bufs=2) as sth,
            tc.tile_pool(name="pg", bufs=4, space="PSUM") as pg,
        ):
            args = (tc, cin, work, stg, sth, pg)
            if reps == 1:
                body(*args)
            else:
                with tc.For_i(0, reps, 1):
                    body(*args)

    nc.compile()
    return nc


def _q8(x):
    return np.asarray(np.asarray(x, np.float32), ml_dtypes.float8_e4m3)


def make_in_maps(batch: np.ndarray):
    E = np.ascontiguousarray(batch.reshape(N, D).astype(np.float32, copy=False))
    n = (E.astype(np.float64) * E).sum(1).astype(np.float32)
    hi = _q8(n).astype(np.float32)
    lo = _q8(n - hi).astype(np.float32)
    lo2 = _q8(n - hi - lo)
    k0_full = _q8(E.T)            # [D, N]
    em_full = _q8(-2.0 * E.T)     # [D, N]

    idx = np.arange(P)
    same = (idx[:, None] // S) == (idx[None, :] // S)
    m8min = np.where(same, BIG, 0.0).astype(np.float32)   # exclude cluster
    m8max = np.where(same, 0.0, -BIG).astype(np.float32)  # keep cluster

    in_maps = []
    for r in range(CORES):
        order = (np.arange(N) + r * M) % N
        et8 = np.zeros((P, 2 * N), dtype=ml_dtypes.float8_e4m3)
        et8[:, 0:N] = k0_full[:, order]
        et8[0, N:2 * N] = _q8(hi[order])
        et8[1, N:2 * N] = _q8(lo[order])
        et8[2, N:2 * N] = lo2[order]
        em8 = np.zeros((P, 2 * M), dtype=ml_dtypes.float8_e4m3)
        em8[:, 0:M] = em_full[:, r * M:(r + 1) * M]
        em8[0:3, M:2 * M] = np.asarray(1.0, ml_dtypes.float8_e4m3)
        nmy = n[r * M:(r + 1) * M].reshape(CH, P).T.copy()   # [P, CH]
        nbias = ((SHIFT - nmy) / TAU).astype(np.float32)
        in_maps.append({"et8": et8, "em8": em8, "m8max": m8max,
                        "m8min": m8min, "nbias": nbias, "nmy": nmy,
                        "onesc": np.ones((P, 2), np.float32)})
    return in_maps


def kernel(batch: np.ndarray) -> np.ndarray:
    if "nc" not in _CACHE:
        _CACHE["nc"] = build_program(reps=1)
    nc = _CACHE["nc"]
    in_maps = make_in_maps(np.asarray(batch))
    res = run_bass_kernel_spmd(nc, in_maps, core_ids=list(range(CORES)))
    total = sum(float(res.results[r]["out"][0, 0]) for r in range(CORES))
    return np.float32(total / N)
